# revision 24
# baseline (speedup 1.0000x reference)
"""Trainium2 Bass kernel for an attention block, data-parallel over batch on 8 NeuronCores.

Reference computation per batch b (softmax over the QUERY axis, axis=1):
    Q = q @ Wq.T + bq            [S, H]
    K = k @ Wk.T + bk            [S, H]
    V = v @ Wv.T + bv            [S, H]
    dot = Q @ K.T                [Sq, Sk]
    W = softmax(dot / sqrt(H), axis=0 over Sq)   (column softmax)
    out = W @ V                  [Sq, H]
    returns (out, W)

Sharding: B=8 batches -> 8 cores, one batch per core; weights replicated.
Host does layout only (transpose/slice/stack); all arithmetic on device.

Device dataflow per core (all contractions need the contracted dim on SBUF
partitions, so inputs are fed pre-transposed):
    QT[h, sq] = sum_d WqT[d, h].T-part @ qT[d, sq]   (float32r matmuls)
    KT[h, sk] likewise; V[sk, h] = vT tiles (stationary) x WvT (moving)
    dotT[sk, sq] = KT-tiles (stationary) x QT (moving)  -> PSUM
    eblk = exp(dotT / 32)  (ScalarE, from PSUM)  [sk_p, sq_f]
    rowsum over free axis (=sum over q) -> reciprocal -> normalize
    WT[sk, sq] written to DRAM (host transposes back); bf16 copy kept for:
    out[sq, h] = sum_sk WT-tiles (stationary) x V (moving)
"""

import sys

if "/opt/trn_rl_repo" not in sys.path:
    sys.path.insert(0, "/opt/trn_rl_repo")

import numpy as np

import concourse.bass as bass
import concourse.mybir as mybir
import concourse.tile as tile
from concourse.tile_rust import add_dep_helper
from concourse.vector_clock import ScopedClock

B, S, D, H = 8, 2048, 1024, 1024
FP32 = mybir.dt.float32
FP32R = mybir.dt.float32r
BF16 = mybir.dt.bfloat16
FP16 = mybir.dt.float16
SCALE = 1.0 / 32.0  # 1/sqrt(H)

N_CORES = 8


class SplitDrainTileContext(tile.TileContext):
    """The walrus build in this container caps sync waits at 1 per
    instruction; Tile can assign several.  Split the extra waits onto
    preceding NoOp instructions on the same engine (program order =>
    identical semantics)."""

    MAX_WAITS = 1

    def _lower_ordered_insts(self, ordered):
        nsplit = 0
        for bb_name, insts in list(ordered.items()):
            new_insts = []
            for inst in insts:
                si = getattr(inst, "sync_info", None)
                if si is not None and si.on_wait and len(si.on_wait) > self.MAX_WAITS:
                    waits = list(si.on_wait)
                    for j, w in enumerate(waits[: -self.MAX_WAITS]):
                        nop = mybir.InstNoOp(name=f"{inst.name}-sw{j}")
                        nop.engine = inst.engine
                        nop.sync_info = mybir.SyncInfo(on_wait=[w], on_update=[])
                        self.nc.register_instruction(nop)
                        new_insts.append(nop)
                        nsplit += 1
                    si.on_wait = waits[-self.MAX_WAITS :]
                new_insts.append(inst)
            ordered[bb_name] = new_insts
        if nsplit:
            print(f"SplitDrainTileContext: split {nsplit} extra sync waits")
        return super()._lower_ordered_insts(ordered)

    def _drain_and_barrier(self, tick_clock, wait_clock):
        drain_inst = self.nc.sync.drain()
        wait_clock.add_sem_waits(
            drain_inst.ins, ScopedClock({None: tick_clock.global_clock})
        )
        si = drain_inst.ins.sync_info
        waits = list(si.on_wait) if si and si.on_wait else []
        if len(waits) > self.MAX_WAITS:
            si.on_wait = waits[: self.MAX_WAITS]
            rest = waits[self.MAX_WAITS :]
            for i in range(0, len(rest), self.MAX_WAITS):
                extra = self.nc.sync.drain()
                extra.ins.sync_info = mybir.SyncInfo(
                    on_wait=rest[i : i + self.MAX_WAITS], on_update=[]
                )
        self.nc.all_engine_barrier()
        assert self.sems is not None
        popped = self.nc._tile_sem_poison_stack.pop()
        assert popped is self._sem_poison
        self.nc.clear_and_free_semaphores(list(self.sems.allocated().values()))
        self.nc.all_engine_barrier()


def build_graph() -> bass.Bass:
    nc = bass.Bass("TRN2", target_bir_lowering=False, debug=False)

    qT_h = nc.declare_dram_parameter("qT", [D, S], FP32, isOutput=False)
    kT_h = nc.declare_dram_parameter("kT", [D, S], FP32, isOutput=False)
    vT_h = nc.declare_dram_parameter("vT", [D, S], FP32, isOutput=False)
    wqT_h = nc.declare_dram_parameter("wqT", [D, H], FP32, isOutput=False)
    wkT_h = nc.declare_dram_parameter("wkT", [D, H], FP32, isOutput=False)
    wvT_h = nc.declare_dram_parameter("wvT", [D, H], FP32, isOutput=False)
    bq_h = nc.declare_dram_parameter("bq", [H], FP32, isOutput=False)
    bk_h = nc.declare_dram_parameter("bk", [H], FP32, isOutput=False)
    bv_h = nc.declare_dram_parameter("bv", [H], FP32, isOutput=False)
    out_h = nc.declare_dram_parameter("out", [S, H], FP32, isOutput=True)
    wT_h = nc.declare_dram_parameter("wT", [S, S], FP32, isOutput=True)

    KD = D // 128  # 8 contraction tiles for d
    MH = H // 128  # 8 h tiles
    NS = S // 512  # 4 sq chunks of 512
    MS = S // 128  # 16 sk tiles
    NH = H // 512  # 2 h chunks of 512

    with SplitDrainTileContext(nc) as tc:
        with (
            tc.tile_pool(name="consts", bufs=1) as consts,
            tc.tile_pool(name="qt", bufs=1) as qt_pool,
            tc.tile_pool(name="kt", bufs=1) as kt_pool,
            tc.tile_pool(name="vv", bufs=1) as v_pool,
            tc.tile_pool(name="psum", bufs=8, space="PSUM") as psum,
        ):
            # ---- constants ----
            bq_sb = consts.tile([128, MH], FP32, tag="bq")
            nc.scalar.dma_start(out=bq_sb, in_=bq_h[:].rearrange("(m p) -> p m", p=128))
            bk_sb = consts.tile([128, MH], FP32, tag="bk")
            nc.scalar.dma_start(out=bk_sb, in_=bk_h[:].rearrange("(m p) -> p m", p=128))
            bvb = consts.tile([128, H], FP16, tag="bvb")
            bv_ap = bv_h[:]
            bv_bcast = bass.AP(
                tensor=bv_ap.tensor,
                offset=bv_ap.offset,
                ap=[[0, 128], *bv_ap.ap],
            )
            nc.gpsimd.dma_start(out=bvb, in_=bv_bcast)

            # ---- persistent activation buffers ----
            QT = [qt_pool.tile([128, S], FP16, tag=f"qt{kk}", name=f"QT{kk}") for kk in range(MH)]
            KT = [kt_pool.tile([128, S], FP16, tag=f"kt{kk}", name=f"KT{kk}") for kk in range(MH)]
            V = [v_pool.tile([128, H], FP16, tag=f"v{m}", name=f"V{m}") for m in range(MS)]

            # ---- phase 1: projections ----
            with (
                tc.tile_pool(name="wpool", bufs=2) as wpool,
                tc.tile_pool(name="wstage", bufs=4) as wstage,
                tc.tile_pool(name="astream", bufs=3) as astream,
            ):
                anchor = [None]  # a mid-projection inst the NEXT projection's
                                 # weight prefetch must not overtake
                for proj, (aT_h, w_h, outT, b_sb) in enumerate(
                    (
                        (qT_h, wqT_h, QT, bq_sb),
                        (kT_h, wkT_h, KT, bk_sb),
                    )
                ):
                    wsb = []
                    if proj == 0:
                        # head: interleave chunk/weight DMA emission per kk so
                        # the first 2-kk burst's deps are the first ~1.5MB of
                        # transfers, not the last of 6MB.
                        ach0 = []
                        for kk in range(KD):
                            at = astream.tile(
                                [128, 512], FP16, tag=f"a{kk}", name=f"ach0_{kk}"
                            )
                            aeng = nc.gpsimd
                            aeng.dma_start(
                                out=at, in_=aT_h[kk * 128 : (kk + 1) * 128, 0:512]
                            )
                            ach0.append(at)
                            wst = wstage.tile([128, H], FP32, tag="wst", name=f"wst_{proj}_{kk}")
                            nc.scalar.dma_start(
                                out=wst, in_=w_h[kk * 128 : (kk + 1) * 128, :]
                            )
                            wt = wpool.tile(
                                [128, H], FP16, tag=f"w{kk}", name=f"w_{proj}_{kk}"
                            )
                            nc.vector.tensor_copy(wt, wst)
                            wsb.append(wt)
                    else:
                        for kk in range(KD):
                            wst = wstage.tile([128, H], FP32, tag="wst", name=f"wst_{proj}_{kk}")
                            wdma = nc.scalar.dma_start(
                                out=wst, in_=w_h[kk * 128 : (kk + 1) * 128, :]
                            )
                            if anchor[0] is not None:
                                add_dep_helper(
                                    wdma.ins, anchor[0], sync=False,
                                    reason="weight prefetch behind head chunks",
                                )
                            wt = wpool.tile(
                                [128, H], FP16, tag=f"w{kk}", name=f"w_{proj}_{kk}"
                            )
                            nc.vector.tensor_copy(wt, wst)
                            wsb.append(wt)
                    for n in range(NS):
                        if proj == 0 and n == 0:
                            ach = ach0
                        else:
                            ach = []
                            for kk in range(KD):
                                at = astream.tile([128, 512], FP16, tag=f"a{kk}")
                                aeng = nc.gpsimd
                                adma = aeng.dma_start(
                                    out=at,
                                    in_=aT_h[
                                        kk * 128 : (kk + 1) * 128, n * 512 : (n + 1) * 512
                                    ],
                                )
                                if n == 1 and kk == KD - 1:
                                    anchor[0] = adma.ins
                                ach.append(at)
                        if proj == 0 and n == 0:
                            # 2-kk bursts: first matmuls need only ach[0..1] +
                            # wq[0..1]; later kk pairs stream in behind.
                            pss = [
                                psum.tile([128, 512], FP32, tag="ps", name=f"hps{m}")
                                for m in range(MH)
                            ]
                            for half in range(KD // 2):
                                for m in range(MH):
                                    for kk in (2 * half, 2 * half + 1):
                                        nc.tensor.matmul(
                                            pss[m],
                                            wsb[kk][:, m * 128 : (m + 1) * 128],
                                            ach[kk],
                                            start=(kk == 0),
                                            stop=(kk == KD - 1),
                                        )
                            for m in range(MH):
                                nc.vector.tensor_scalar_add(
                                    outT[m][:, n * 512 : (n + 1) * 512],
                                    pss[m],
                                    b_sb[:, m : m + 1],
                                )
                            continue
                        for m in range(MH):
                            ps = psum.tile([128, 512], FP32, tag="ps")
                            for kk in range(KD):
                                nc.tensor.matmul(
                                    ps,
                                    wsb[kk][:, m * 128 : (m + 1) * 128],
                                    ach[kk],
                                    start=(kk == 0),
                                    stop=(kk == KD - 1),
                                )
                            nc.vector.tensor_scalar_add(
                                outT[m][:, n * 512 : (n + 1) * 512],
                                ps,
                                b_sb[:, m : m + 1],
                            )

                # V projection: stationary = vT chunk slices, moving = WvT
                wsb = []
                for kk in range(KD):
                    wst = wstage.tile([128, H], FP32, tag="wst", name=f"wst_v_{kk}")
                    wdma = nc.scalar.dma_start(out=wst, in_=wvT_h[kk * 128 : (kk + 1) * 128, :])
                    if anchor[0] is not None:
                        add_dep_helper(
                            wdma.ins, anchor[0], sync=False,
                            reason="wv prefetch behind k head chunks",
                        )
                    wt = wpool.tile([128, H], FP16, tag=f"w{kk}", name=f"w_v_{kk}")
                    nc.vector.tensor_copy(wt, wst)
                    wsb.append(wt)
                for g in range(MS // 4):  # groups of 4 sk tiles = 512 cols of vT
                    vch = []
                    for kk in range(KD):
                        vt = astream.tile([128, 512], FP16, tag=f"a{kk}", name=f"vch_{g}_{kk}")
                        veng = nc.gpsimd
                        veng.dma_start(
                            out=vt,
                            in_=vT_h[
                                kk * 128 : (kk + 1) * 128, g * 512 : (g + 1) * 512
                            ],
                        )
                        vch.append(vt)
                    for mloc in range(4):
                        m = g * 4 + mloc
                        for n in range(NH):
                            ps = psum.tile([128, 512], FP32, tag="ps")
                            for kk in range(KD):
                                nc.tensor.matmul(
                                    ps,
                                    vch[kk][:, mloc * 128 : (mloc + 1) * 128],
                                    wsb[kk][:, n * 512 : (n + 1) * 512],
                                    start=(kk == 0),
                                    stop=(kk == KD - 1),
                                )
                            nc.vector.tensor_tensor(
                                V[m][:, n * 512 : (n + 1) * 512],
                                ps,
                                bvb[:, n * 512 : (n + 1) * 512],
                                mybir.AluOpType.add,
                            )

            # ---- phase 2+3: attention ----
            with (
                tc.tile_pool(name="wt", bufs=1) as wt_pool,
                tc.tile_pool(name="eblk", bufs=2) as epool,
                tc.tile_pool(name="sums", bufs=8) as spool,
                tc.tile_pool(name="ostage", bufs=4) as opool,
            ):
                WT = [wt_pool.tile([128, S], FP16, tag=f"wt{m}", name=f"WT{m}") for m in range(MS)]

                for m in range(MS):
                    eb = epool.tile([128, S], FP32, tag="eb")
                    for n in range(NS):
                        ps = psum.tile([128, 512], FP32, tag="ps")
                        for kk in range(MH):
                            nc.tensor.matmul(
                                ps,
                                KT[kk][:, m * 128 : (m + 1) * 128],
                                QT[kk][:, n * 512 : (n + 1) * 512],
                                start=(kk == 0),
                                stop=(kk == MH - 1),
                            )
                        nc.scalar.activation(
                            out=eb[:, n * 512 : (n + 1) * 512],
                            in_=ps,
                            func=mybir.ActivationFunctionType.Exp,
                            scale=SCALE,
                        )
                    ssum = spool.tile([128, 1], FP32, tag="s")
                    nc.vector.tensor_reduce(
                        out=ssum, in_=eb, axis=mybir.AxisListType.X,
                        op=mybir.AluOpType.add,
                    )
                    rcp = spool.tile([128, 1], FP32, tag="r")
                    nc.vector.reciprocal(rcp, ssum)
                    # bf16 normalized copy for the second bmm
                    nc.scalar.activation(
                        out=WT[m],
                        in_=eb,
                        func=mybir.ActivationFunctionType.Copy,
                        scale=rcp,
                    )
                    # f32 normalized row block -> DRAM (transposed attn_weights)
                    nc.vector.tensor_scalar_mul(eb, eb, rcp)
                    nc.gpsimd.dma_start(out=wT_h[m * 128 : (m + 1) * 128, :], in_=eb)

                # out tiles in groups of 8 PSUM banks with the sk-block loop (b)
                # hoisted to the middle: the WT[15]-dependent matmuls sit ~120
                # instructions deep, hiding the last softmax block's latency.
                otiles = [(mm, n) for mm in range(MS) for n in range(NH)]
                bounds = list(range(0, 28, 4)) + [28, 30]
                for gi, gstart in enumerate(bounds):
                    gend = bounds[gi + 1] if gi + 1 < len(bounds) else len(otiles)
                    group = otiles[gstart:gend]
                    g = gstart
                    pss = [
                        psum.tile([128, 512], FP32, tag="ps", name=f"ops_{g}_{j}")
                        for j in range(len(group))
                    ]
                    for b in range(MS):
                        for j, (mm, n) in enumerate(group):
                            nc.tensor.matmul(
                                pss[j],
                                WT[b][:, mm * 128 : (mm + 1) * 128],
                                V[b][:, n * 512 : (n + 1) * 512],
                                start=(b == 0),
                                stop=(b == MS - 1),
                            )
                    for j, (mm, n) in enumerate(group):
                        ost = opool.tile([128, 512], FP32, tag="o")
                        if j % 2 == 0:
                            nc.vector.tensor_copy(ost, pss[j])
                        else:
                            nc.scalar.activation(
                                out=ost, in_=pss[j],
                                func=mybir.ActivationFunctionType.Copy,
                            )
                        nc.scalar.dma_start(
                            out=out_h[mm * 128 : (mm + 1) * 128, n * 512 : (n + 1) * 512],
                            in_=ost,
                        )

    return nc


_GRAPH_CACHE = {}


def get_graph() -> bass.Bass:
    if "nc" not in _GRAPH_CACHE:
        _GRAPH_CACHE["nc"] = build_graph()
    return _GRAPH_CACHE["nc"]


def kernel(q, k, v, Wq, bq, Wk, bk, Wv, bv):
    from concourse.bass_utils import run_bass_kernel_spmd

    assert q.shape == (B, S, D)
    nc = get_graph()

    qT = np.ascontiguousarray(q.transpose(0, 2, 1), dtype=np.float32)
    kT = np.ascontiguousarray(k.transpose(0, 2, 1), dtype=np.float32)
    vT = np.ascontiguousarray(v.transpose(0, 2, 1), dtype=np.float32)
    wqT = np.ascontiguousarray(Wq.T, dtype=np.float32)
    wkT = np.ascontiguousarray(Wk.T, dtype=np.float32)
    wvT = np.ascontiguousarray(Wv.T, dtype=np.float32)
    bq = np.ascontiguousarray(bq, dtype=np.float32)
    bk = np.ascontiguousarray(bk, dtype=np.float32)
    bv = np.ascontiguousarray(bv, dtype=np.float32)

    in_maps = [
        dict(
            qT=qT[i], kT=kT[i], vT=vT[i],
            wqT=wqT, wkT=wkT, wvT=wvT,
            bq=bq, bk=bk, bv=bv,
        )
        for i in range(N_CORES)
    ]
    res = run_bass_kernel_spmd(nc, in_maps, core_ids=list(range(N_CORES)))
    attn_outputs = np.stack([res.results[i]["out"] for i in range(N_CORES)])
    wT = np.stack([res.results[i]["wT"] for i in range(N_CORES)])
    attn_weights = np.ascontiguousarray(wT.transpose(0, 2, 1))
    return attn_outputs, attn_weights


# revision 25
# speedup vs baseline: 1.0172x; 1.0172x over previous
"""Trainium2 Bass kernel for an attention block, data-parallel over batch on 8 NeuronCores.

Reference computation per batch b (softmax over the QUERY axis, axis=1):
    Q = q @ Wq.T + bq            [S, H]
    K = k @ Wk.T + bk            [S, H]
    V = v @ Wv.T + bv            [S, H]
    dot = Q @ K.T                [Sq, Sk]
    W = softmax(dot / sqrt(H), axis=0 over Sq)   (column softmax)
    out = W @ V                  [Sq, H]
    returns (out, W)

Sharding: B=8 batches -> 8 cores, one batch per core; weights replicated.
Host does layout only (transpose/slice/stack); all arithmetic on device.

Device dataflow per core (all contractions need the contracted dim on SBUF
partitions, so inputs are fed pre-transposed):
    QT[h, sq] = sum_d WqT[d, h].T-part @ qT[d, sq]   (float32r matmuls)
    KT[h, sk] likewise; V[sk, h] = vT tiles (stationary) x WvT (moving)
    dotT[sk, sq] = KT-tiles (stationary) x QT (moving)  -> PSUM
    eblk = exp(dotT / 32)  (ScalarE, from PSUM)  [sk_p, sq_f]
    rowsum over free axis (=sum over q) -> reciprocal -> normalize
    WT[sk, sq] written to DRAM (host transposes back); bf16 copy kept for:
    out[sq, h] = sum_sk WT-tiles (stationary) x V (moving)
"""

import sys

if "/opt/trn_rl_repo" not in sys.path:
    sys.path.insert(0, "/opt/trn_rl_repo")

import numpy as np

import concourse.bass as bass
import concourse.mybir as mybir
import concourse.tile as tile
from concourse.tile_rust import add_dep_helper
from concourse.vector_clock import ScopedClock

B, S, D, H = 8, 2048, 1024, 1024
FP32 = mybir.dt.float32
FP32R = mybir.dt.float32r
BF16 = mybir.dt.bfloat16
FP16 = mybir.dt.float16
SCALE = 1.0 / 32.0  # 1/sqrt(H)

N_CORES = 8


class SplitDrainTileContext(tile.TileContext):
    """The walrus build in this container caps sync waits at 1 per
    instruction; Tile can assign several.  Split the extra waits onto
    preceding NoOp instructions on the same engine (program order =>
    identical semantics)."""

    MAX_WAITS = 1

    def _lower_ordered_insts(self, ordered):
        nsplit = 0
        for bb_name, insts in list(ordered.items()):
            new_insts = []
            for inst in insts:
                si = getattr(inst, "sync_info", None)
                if si is not None and si.on_wait and len(si.on_wait) > self.MAX_WAITS:
                    waits = list(si.on_wait)
                    for j, w in enumerate(waits[: -self.MAX_WAITS]):
                        nop = mybir.InstNoOp(name=f"{inst.name}-sw{j}")
                        nop.engine = inst.engine
                        nop.sync_info = mybir.SyncInfo(on_wait=[w], on_update=[])
                        self.nc.register_instruction(nop)
                        new_insts.append(nop)
                        nsplit += 1
                    si.on_wait = waits[-self.MAX_WAITS :]
                new_insts.append(inst)
            ordered[bb_name] = new_insts
        if nsplit:
            print(f"SplitDrainTileContext: split {nsplit} extra sync waits")
        return super()._lower_ordered_insts(ordered)

    def _drain_and_barrier(self, tick_clock, wait_clock):
        drain_inst = self.nc.sync.drain()
        wait_clock.add_sem_waits(
            drain_inst.ins, ScopedClock({None: tick_clock.global_clock})
        )
        si = drain_inst.ins.sync_info
        waits = list(si.on_wait) if si and si.on_wait else []
        if len(waits) > self.MAX_WAITS:
            si.on_wait = waits[: self.MAX_WAITS]
            rest = waits[self.MAX_WAITS :]
            for i in range(0, len(rest), self.MAX_WAITS):
                extra = self.nc.sync.drain()
                extra.ins.sync_info = mybir.SyncInfo(
                    on_wait=rest[i : i + self.MAX_WAITS], on_update=[]
                )
        self.nc.all_engine_barrier()
        assert self.sems is not None
        popped = self.nc._tile_sem_poison_stack.pop()
        assert popped is self._sem_poison
        self.nc.clear_and_free_semaphores(list(self.sems.allocated().values()))
        self.nc.all_engine_barrier()


def build_graph() -> bass.Bass:
    nc = bass.Bass("TRN2", target_bir_lowering=False, debug=False)

    qT_h = nc.declare_dram_parameter("qT", [D, S], FP32, isOutput=False)
    kT_h = nc.declare_dram_parameter("kT", [D, S], FP32, isOutput=False)
    vT_h = nc.declare_dram_parameter("vT", [D, S], FP32, isOutput=False)
    wqT_h = nc.declare_dram_parameter("wqT", [D, H], FP32, isOutput=False)
    wkT_h = nc.declare_dram_parameter("wkT", [D, H], FP32, isOutput=False)
    wvT_h = nc.declare_dram_parameter("wvT", [D, H], FP32, isOutput=False)
    bq_h = nc.declare_dram_parameter("bq", [H], FP32, isOutput=False)
    bk_h = nc.declare_dram_parameter("bk", [H], FP32, isOutput=False)
    bv_h = nc.declare_dram_parameter("bv", [H], FP32, isOutput=False)
    out_h = nc.declare_dram_parameter("out", [S, H], FP32, isOutput=True)
    wT_h = nc.declare_dram_parameter("wT", [S, S], FP32, isOutput=True)

    KD = D // 128  # 8 contraction tiles for d
    MH = H // 128  # 8 h tiles
    NS = S // 512  # 4 sq chunks of 512
    MS = S // 128  # 16 sk tiles
    NH = H // 512  # 2 h chunks of 512

    with SplitDrainTileContext(nc) as tc:
        with (
            tc.tile_pool(name="consts", bufs=1) as consts,
            tc.tile_pool(name="qt", bufs=1) as qt_pool,
            tc.tile_pool(name="kt", bufs=1) as kt_pool,
            tc.tile_pool(name="vv", bufs=1) as v_pool,
            tc.tile_pool(name="psum", bufs=8, space="PSUM") as psum,
        ):
            # ---- constants ----
            bq_sb = consts.tile([128, MH], FP32, tag="bq")
            nc.scalar.dma_start(out=bq_sb, in_=bq_h[:].rearrange("(m p) -> p m", p=128))
            bk_sb = consts.tile([128, MH], FP32, tag="bk")
            nc.scalar.dma_start(out=bk_sb, in_=bk_h[:].rearrange("(m p) -> p m", p=128))
            bvb = consts.tile([128, H], FP16, tag="bvb")
            bv_ap = bv_h[:]
            bv_bcast = bass.AP(
                tensor=bv_ap.tensor,
                offset=bv_ap.offset,
                ap=[[0, 128], *bv_ap.ap],
            )
            nc.gpsimd.dma_start(out=bvb, in_=bv_bcast)

            # ---- persistent activation buffers ----
            QT = [qt_pool.tile([128, S], FP16, tag=f"qt{kk}", name=f"QT{kk}") for kk in range(MH)]
            KT = [kt_pool.tile([128, S], FP16, tag=f"kt{kk}", name=f"KT{kk}") for kk in range(MH)]
            V = [v_pool.tile([128, H], FP16, tag=f"v{m}", name=f"V{m}") for m in range(MS)]

            # ---- phase 1: projections ----
            with (
                tc.tile_pool(name="wpool", bufs=2) as wpool,
                tc.tile_pool(name="astream", bufs=3) as astream,
            ):
                anchor = [None]  # a mid-projection inst the NEXT projection's
                                 # weight prefetch must not overtake
                for proj, (aT_h, w_h, outT, b_sb) in enumerate(
                    (
                        (qT_h, wqT_h, QT, bq_sb),
                        (kT_h, wkT_h, KT, bk_sb),
                    )
                ):
                    wsb = []
                    if proj == 0:
                        # head: interleave chunk/weight DMA emission per kk so
                        # the first 2-kk burst's deps are the first ~1.5MB of
                        # transfers, not the last of 6MB.
                        ach0 = []
                        for kk in range(KD):
                            at = astream.tile(
                                [128, 512], FP16, tag=f"a{kk}", name=f"ach0_{kk}"
                            )
                            aeng = nc.gpsimd
                            aeng.dma_start(
                                out=at, in_=aT_h[kk * 128 : (kk + 1) * 128, 0:512]
                            )
                            ach0.append(at)
                            wt = wpool.tile(
                                [128, H], FP16, tag=f"w{kk}", name=f"w_{proj}_{kk}"
                            )
                            weng = nc.gpsimd
                            weng.dma_start(
                                out=wt, in_=w_h[kk * 128 : (kk + 1) * 128, :]
                            )
                            wsb.append(wt)
                    else:
                        for kk in range(KD):
                            wt = wpool.tile(
                                [128, H], FP16, tag=f"w{kk}", name=f"w_{proj}_{kk}"
                            )
                            weng = nc.gpsimd
                            wdma = weng.dma_start(
                                out=wt, in_=w_h[kk * 128 : (kk + 1) * 128, :]
                            )
                            if anchor[0] is not None:
                                add_dep_helper(
                                    wdma.ins, anchor[0], sync=False,
                                    reason="weight prefetch behind head chunks",
                                )
                            wsb.append(wt)
                    for n in range(NS):
                        if proj == 0 and n == 0:
                            ach = ach0
                        else:
                            ach = []
                            for kk in range(KD):
                                at = astream.tile([128, 512], FP16, tag=f"a{kk}")
                                aeng = nc.gpsimd
                                adma = aeng.dma_start(
                                    out=at,
                                    in_=aT_h[
                                        kk * 128 : (kk + 1) * 128, n * 512 : (n + 1) * 512
                                    ],
                                )
                                if n == 1 and kk == KD - 1:
                                    anchor[0] = adma.ins
                                ach.append(at)
                        if proj == 0 and n == 0:
                            # 2-kk bursts: first matmuls need only ach[0..1] +
                            # wq[0..1]; later kk pairs stream in behind.
                            pss = [
                                psum.tile([128, 512], FP32, tag="ps", name=f"hps{m}")
                                for m in range(MH)
                            ]
                            for half in range(KD // 2):
                                for m in range(MH):
                                    for kk in (2 * half, 2 * half + 1):
                                        nc.tensor.matmul(
                                            pss[m],
                                            wsb[kk][:, m * 128 : (m + 1) * 128],
                                            ach[kk],
                                            start=(kk == 0),
                                            stop=(kk == KD - 1),
                                        )
                            for m in range(MH):
                                nc.vector.tensor_scalar_add(
                                    outT[m][:, n * 512 : (n + 1) * 512],
                                    pss[m],
                                    b_sb[:, m : m + 1],
                                )
                            continue
                        for m in range(MH):
                            ps = psum.tile([128, 512], FP32, tag="ps")
                            for kk in range(KD):
                                nc.tensor.matmul(
                                    ps,
                                    wsb[kk][:, m * 128 : (m + 1) * 128],
                                    ach[kk],
                                    start=(kk == 0),
                                    stop=(kk == KD - 1),
                                )
                            nc.vector.tensor_scalar_add(
                                outT[m][:, n * 512 : (n + 1) * 512],
                                ps,
                                b_sb[:, m : m + 1],
                            )

                # V projection: stationary = vT chunk slices, moving = WvT
                wsb = []
                for kk in range(KD):
                    wt = wpool.tile([128, H], FP16, tag=f"w{kk}", name=f"w_v_{kk}")
                    weng = nc.gpsimd
                    wdma = weng.dma_start(out=wt, in_=wvT_h[kk * 128 : (kk + 1) * 128, :])
                    if anchor[0] is not None:
                        add_dep_helper(
                            wdma.ins, anchor[0], sync=False,
                            reason="wv prefetch behind k head chunks",
                        )
                    wsb.append(wt)
                for g in range(MS // 4):  # groups of 4 sk tiles = 512 cols of vT
                    vch = []
                    for kk in range(KD):
                        vt = astream.tile([128, 512], FP16, tag=f"a{kk}", name=f"vch_{g}_{kk}")
                        veng = nc.gpsimd
                        veng.dma_start(
                            out=vt,
                            in_=vT_h[
                                kk * 128 : (kk + 1) * 128, g * 512 : (g + 1) * 512
                            ],
                        )
                        vch.append(vt)
                    for mloc in range(4):
                        m = g * 4 + mloc
                        for n in range(NH):
                            ps = psum.tile([128, 512], FP32, tag="ps")
                            for kk in range(KD):
                                nc.tensor.matmul(
                                    ps,
                                    vch[kk][:, mloc * 128 : (mloc + 1) * 128],
                                    wsb[kk][:, n * 512 : (n + 1) * 512],
                                    start=(kk == 0),
                                    stop=(kk == KD - 1),
                                )
                            nc.vector.tensor_tensor(
                                V[m][:, n * 512 : (n + 1) * 512],
                                ps,
                                bvb[:, n * 512 : (n + 1) * 512],
                                mybir.AluOpType.add,
                            )

            # ---- phase 2+3: attention ----
            with (
                tc.tile_pool(name="wt", bufs=1) as wt_pool,
                tc.tile_pool(name="eblk", bufs=2) as epool,
                tc.tile_pool(name="sums", bufs=8) as spool,
                tc.tile_pool(name="ostage", bufs=4) as opool,
            ):
                WT = [wt_pool.tile([128, S], FP16, tag=f"wt{m}", name=f"WT{m}") for m in range(MS)]

                for m in range(MS):
                    eb = epool.tile([128, S], FP32, tag="eb")
                    for n in range(NS):
                        ps = psum.tile([128, 512], FP32, tag="ps")
                        for kk in range(MH):
                            nc.tensor.matmul(
                                ps,
                                KT[kk][:, m * 128 : (m + 1) * 128],
                                QT[kk][:, n * 512 : (n + 1) * 512],
                                start=(kk == 0),
                                stop=(kk == MH - 1),
                            )
                        nc.scalar.activation(
                            out=eb[:, n * 512 : (n + 1) * 512],
                            in_=ps,
                            func=mybir.ActivationFunctionType.Exp,
                            scale=SCALE,
                        )
                    ssum = spool.tile([128, 1], FP32, tag="s")
                    nc.vector.tensor_reduce(
                        out=ssum, in_=eb, axis=mybir.AxisListType.X,
                        op=mybir.AluOpType.add,
                    )
                    rcp = spool.tile([128, 1], FP32, tag="r")
                    nc.vector.reciprocal(rcp, ssum)
                    # bf16 normalized copy for the second bmm
                    nc.scalar.activation(
                        out=WT[m],
                        in_=eb,
                        func=mybir.ActivationFunctionType.Copy,
                        scale=rcp,
                    )
                    # f32 normalized row block -> DRAM (transposed attn_weights)
                    nc.vector.tensor_scalar_mul(eb, eb, rcp)
                    nc.gpsimd.dma_start(out=wT_h[m * 128 : (m + 1) * 128, :], in_=eb)

                # out tiles in groups of 8 PSUM banks with the sk-block loop (b)
                # hoisted to the middle: the WT[15]-dependent matmuls sit ~120
                # instructions deep, hiding the last softmax block's latency.
                otiles = [(mm, n) for mm in range(MS) for n in range(NH)]
                bounds = list(range(0, 28, 4)) + [28, 30]
                for gi, gstart in enumerate(bounds):
                    gend = bounds[gi + 1] if gi + 1 < len(bounds) else len(otiles)
                    group = otiles[gstart:gend]
                    g = gstart
                    pss = [
                        psum.tile([128, 512], FP32, tag="ps", name=f"ops_{g}_{j}")
                        for j in range(len(group))
                    ]
                    for b in range(MS):
                        for j, (mm, n) in enumerate(group):
                            nc.tensor.matmul(
                                pss[j],
                                WT[b][:, mm * 128 : (mm + 1) * 128],
                                V[b][:, n * 512 : (n + 1) * 512],
                                start=(b == 0),
                                stop=(b == MS - 1),
                            )
                    for j, (mm, n) in enumerate(group):
                        ost = opool.tile([128, 512], FP32, tag="o")
                        if j % 2 == 0:
                            nc.vector.tensor_copy(ost, pss[j])
                        else:
                            nc.scalar.activation(
                                out=ost, in_=pss[j],
                                func=mybir.ActivationFunctionType.Copy,
                            )
                        nc.scalar.dma_start(
                            out=out_h[mm * 128 : (mm + 1) * 128, n * 512 : (n + 1) * 512],
                            in_=ost,
                        )

    return nc


_GRAPH_CACHE = {}


def get_graph() -> bass.Bass:
    if "nc" not in _GRAPH_CACHE:
        _GRAPH_CACHE["nc"] = build_graph()
    return _GRAPH_CACHE["nc"]


def kernel(q, k, v, Wq, bq, Wk, bk, Wv, bv):
    from concourse.bass_utils import run_bass_kernel_spmd

    assert q.shape == (B, S, D)
    nc = get_graph()

    qT = np.ascontiguousarray(q.transpose(0, 2, 1), dtype=np.float32)
    kT = np.ascontiguousarray(k.transpose(0, 2, 1), dtype=np.float32)
    vT = np.ascontiguousarray(v.transpose(0, 2, 1), dtype=np.float32)
    wqT = np.ascontiguousarray(Wq.T, dtype=np.float32)
    wkT = np.ascontiguousarray(Wk.T, dtype=np.float32)
    wvT = np.ascontiguousarray(Wv.T, dtype=np.float32)
    bq = np.ascontiguousarray(bq, dtype=np.float32)
    bk = np.ascontiguousarray(bk, dtype=np.float32)
    bv = np.ascontiguousarray(bv, dtype=np.float32)

    in_maps = [
        dict(
            qT=qT[i], kT=kT[i], vT=vT[i],
            wqT=wqT, wkT=wkT, wvT=wvT,
            bq=bq, bk=bk, bv=bv,
        )
        for i in range(N_CORES)
    ]
    res = run_bass_kernel_spmd(nc, in_maps, core_ids=list(range(N_CORES)))
    attn_outputs = np.stack([res.results[i]["out"] for i in range(N_CORES)])
    wT = np.stack([res.results[i]["wT"] for i in range(N_CORES)])
    attn_weights = np.ascontiguousarray(wT.transpose(0, 2, 1))
    return attn_outputs, attn_weights


# revision 27
# speedup vs baseline: 1.0265x; 1.0091x over previous
"""Trainium2 Bass kernel for an attention block, data-parallel over batch on 8 NeuronCores.

Reference computation per batch b (softmax over the QUERY axis, axis=1):
    Q = q @ Wq.T + bq            [S, H]
    K = k @ Wk.T + bk            [S, H]
    V = v @ Wv.T + bv            [S, H]
    dot = Q @ K.T                [Sq, Sk]
    W = softmax(dot / sqrt(H), axis=0 over Sq)   (column softmax)
    out = W @ V                  [Sq, H]
    returns (out, W)

Sharding: B=8 batches -> 8 cores, one batch per core; weights replicated.
Host does layout only (transpose/slice/stack); all arithmetic on device.

Device dataflow per core (all contractions need the contracted dim on SBUF
partitions, so inputs are fed pre-transposed; all matmuls run in fp16 at the
full 1-column/cycle PE rate, f32 inputs converted by gpsimd cast-DMAs):
    QT[h, sq] = sum_d WqT[d, h].T-part @ qT[d, sq]
    KT[h, sk] likewise; V[sk, h] = vT tiles (stationary) x WvT (moving)
    dotT[sk, sq] = KT-tiles (stationary) x QT (moving)  -> PSUM
    eblk = exp(dotT / 32)  (ScalarE, from PSUM)  [sk_p, sq_f]
    rowsum over free axis (=sum over q) -> reciprocal -> normalize
    WT[sk, sq] written to DRAM f32 (host transposes back); fp16 copy kept for:
    out[sq, h] = sum_sk WT-tiles (stationary) x V (moving), b-hoisted in
    4-bank PSUM groups so the last softmax block's latency is hidden.
"""

import sys

if "/opt/trn_rl_repo" not in sys.path:
    sys.path.insert(0, "/opt/trn_rl_repo")

import numpy as np

import concourse.bass as bass
import concourse.mybir as mybir
import concourse.tile as tile
from concourse.tile_rust import add_dep_helper
from concourse.vector_clock import ScopedClock

B, S, D, H = 8, 2048, 1024, 1024
FP32 = mybir.dt.float32
FP32R = mybir.dt.float32r
BF16 = mybir.dt.bfloat16
FP16 = mybir.dt.float16
SCALE = 1.0 / 32.0  # 1/sqrt(H)

N_CORES = 8


class SplitDrainTileContext(tile.TileContext):
    """The walrus build in this container caps sync waits at 1 per
    instruction; Tile can assign several.  Split the extra waits onto
    preceding NoOp instructions on the same engine (program order =>
    identical semantics)."""

    MAX_WAITS = 1

    def _lower_ordered_insts(self, ordered):
        nsplit = 0
        for bb_name, insts in list(ordered.items()):
            new_insts = []
            for inst in insts:
                si = getattr(inst, "sync_info", None)
                if si is not None and si.on_wait and len(si.on_wait) > self.MAX_WAITS:
                    waits = list(si.on_wait)
                    for j, w in enumerate(waits[: -self.MAX_WAITS]):
                        nop = mybir.InstNoOp(name=f"{inst.name}-sw{j}")
                        nop.engine = inst.engine
                        nop.sync_info = mybir.SyncInfo(on_wait=[w], on_update=[])
                        self.nc.register_instruction(nop)
                        new_insts.append(nop)
                        nsplit += 1
                    si.on_wait = waits[-self.MAX_WAITS :]
                new_insts.append(inst)
            ordered[bb_name] = new_insts
        if nsplit:
            print(f"SplitDrainTileContext: split {nsplit} extra sync waits")
        return super()._lower_ordered_insts(ordered)

    def _drain_and_barrier(self, tick_clock, wait_clock):
        drain_inst = self.nc.sync.drain()
        wait_clock.add_sem_waits(
            drain_inst.ins, ScopedClock({None: tick_clock.global_clock})
        )
        si = drain_inst.ins.sync_info
        waits = list(si.on_wait) if si and si.on_wait else []
        if len(waits) > self.MAX_WAITS:
            si.on_wait = waits[: self.MAX_WAITS]
            rest = waits[self.MAX_WAITS :]
            for i in range(0, len(rest), self.MAX_WAITS):
                extra = self.nc.sync.drain()
                extra.ins.sync_info = mybir.SyncInfo(
                    on_wait=rest[i : i + self.MAX_WAITS], on_update=[]
                )
        self.nc.all_engine_barrier()
        assert self.sems is not None
        popped = self.nc._tile_sem_poison_stack.pop()
        assert popped is self._sem_poison
        self.nc.clear_and_free_semaphores(list(self.sems.allocated().values()))
        self.nc.all_engine_barrier()


def build_graph() -> bass.Bass:
    nc = bass.Bass("TRN2", target_bir_lowering=False, debug=False)

    qT_h = nc.declare_dram_parameter("qT", [D, S], FP32, isOutput=False)
    kT_h = nc.declare_dram_parameter("kT", [D, S], FP32, isOutput=False)
    vT_h = nc.declare_dram_parameter("vT", [D, S], FP32, isOutput=False)
    wqT_h = nc.declare_dram_parameter("wqT", [D, H], FP32, isOutput=False)
    wkT_h = nc.declare_dram_parameter("wkT", [D, H], FP32, isOutput=False)
    wvT_h = nc.declare_dram_parameter("wvT", [D, H], FP32, isOutput=False)
    bq_h = nc.declare_dram_parameter("bq", [H], FP32, isOutput=False)
    bk_h = nc.declare_dram_parameter("bk", [H], FP32, isOutput=False)
    bv_h = nc.declare_dram_parameter("bv", [H], FP32, isOutput=False)
    out_h = nc.declare_dram_parameter("out", [S, H], FP32, isOutput=True)
    wT_h = nc.declare_dram_parameter("wT", [S, S], FP32, isOutput=True)

    KD = D // 128  # 8 contraction tiles for d
    MH = H // 128  # 8 h tiles
    NS = S // 512  # 4 sq chunks of 512
    MS = S // 128  # 16 sk tiles
    NH = H // 512  # 2 h chunks of 512

    with SplitDrainTileContext(nc) as tc:
        with (
            tc.tile_pool(name="consts", bufs=1) as consts,
            tc.tile_pool(name="qt", bufs=1) as qt_pool,
            tc.tile_pool(name="kt", bufs=1) as kt_pool,
            tc.tile_pool(name="vv", bufs=1) as v_pool,
            tc.tile_pool(name="psum", bufs=8, space="PSUM") as psum,
        ):
            # ---- constants ----
            bq_sb = consts.tile([128, MH], FP32, tag="bq")
            nc.scalar.dma_start(out=bq_sb, in_=bq_h[:].rearrange("(m p) -> p m", p=128))
            bk_sb = consts.tile([128, MH], FP32, tag="bk")
            nc.scalar.dma_start(out=bk_sb, in_=bk_h[:].rearrange("(m p) -> p m", p=128))
            bvb = consts.tile([128, H], FP16, tag="bvb")

            # ---- persistent activation buffers ----
            QT = [qt_pool.tile([128, S], FP16, tag=f"qt{kk}", name=f"QT{kk}") for kk in range(MH)]
            KT = [kt_pool.tile([128, S], FP16, tag=f"kt{kk}", name=f"KT{kk}") for kk in range(MH)]
            V = [v_pool.tile([128, H], FP16, tag=f"v{m}", name=f"V{m}") for m in range(MS)]

            # ---- phase 1: projections ----
            with (
                tc.tile_pool(name="wpool", bufs=2) as wpool,
                tc.tile_pool(name="astream", bufs=3) as astream,
            ):
                anchor = [None]  # a mid-projection inst the NEXT projection's
                                 # weight prefetch must not overtake
                for proj, (aT_h, w_h, outT, b_sb) in enumerate(
                    (
                        (qT_h, wqT_h, QT, bq_sb),
                        (kT_h, wkT_h, KT, bk_sb),
                    )
                ):
                    wsb = []
                    if proj == 0:
                        # head: interleave chunk/weight DMA emission per kk so
                        # the first 2-kk burst's deps are the first ~1.5MB of
                        # transfers, not the last of 6MB.
                        ach0 = []
                        for kk in range(KD):
                            at = astream.tile(
                                [128, 512], FP16, tag=f"a{kk}", name=f"ach0_{kk}"
                            )
                            aeng = nc.gpsimd
                            aeng.dma_start(
                                out=at, in_=aT_h[kk * 128 : (kk + 1) * 128, 0:512]
                            )
                            ach0.append(at)
                            wt = wpool.tile(
                                [128, H], FP16, tag=f"w{kk}", name=f"w_{proj}_{kk}"
                            )
                            weng = nc.gpsimd
                            weng.dma_start(
                                out=wt, in_=w_h[kk * 128 : (kk + 1) * 128, :]
                            )
                            wsb.append(wt)
                    else:
                        for kk in range(KD):
                            wt = wpool.tile(
                                [128, H], FP16, tag=f"w{kk}", name=f"w_{proj}_{kk}"
                            )
                            weng = nc.gpsimd
                            wdma = weng.dma_start(
                                out=wt, in_=w_h[kk * 128 : (kk + 1) * 128, :]
                            )
                            if anchor[0] is not None:
                                add_dep_helper(
                                    wdma.ins, anchor[0], sync=False,
                                    reason="weight prefetch behind head chunks",
                                )
                            wsb.append(wt)
                    for n in range(NS):
                        if proj == 0 and n == 0:
                            ach = ach0
                        else:
                            ach = []
                            for kk in range(KD):
                                at = astream.tile([128, 512], FP16, tag=f"a{kk}")
                                aeng = nc.gpsimd
                                adma = aeng.dma_start(
                                    out=at,
                                    in_=aT_h[
                                        kk * 128 : (kk + 1) * 128, n * 512 : (n + 1) * 512
                                    ],
                                )
                                if n == 1 and kk == KD - 1:
                                    anchor[0] = adma.ins
                                ach.append(at)
                        if proj == 0 and n == 0:
                            # 2-kk bursts: first matmuls need only ach[0..1] +
                            # wq[0..1]; later kk pairs stream in behind.
                            pss = [
                                psum.tile([128, 512], FP32, tag="ps", name=f"hps{m}")
                                for m in range(MH)
                            ]
                            for half in range(KD // 2):
                                for m in range(MH):
                                    for kk in (2 * half, 2 * half + 1):
                                        nc.tensor.matmul(
                                            pss[m],
                                            wsb[kk][:, m * 128 : (m + 1) * 128],
                                            ach[kk],
                                            start=(kk == 0),
                                            stop=(kk == KD - 1),
                                        )
                            for m in range(MH):
                                nc.vector.tensor_scalar_add(
                                    outT[m][:, n * 512 : (n + 1) * 512],
                                    pss[m],
                                    b_sb[:, m : m + 1],
                                )
                            continue
                        for m in range(MH):
                            ps = psum.tile([128, 512], FP32, tag="ps")
                            for kk in range(KD):
                                nc.tensor.matmul(
                                    ps,
                                    wsb[kk][:, m * 128 : (m + 1) * 128],
                                    ach[kk],
                                    start=(kk == 0),
                                    stop=(kk == KD - 1),
                                )
                            nc.vector.tensor_scalar_add(
                                outT[m][:, n * 512 : (n + 1) * 512],
                                ps,
                                b_sb[:, m : m + 1],
                            )

                # V projection: stationary = vT chunk slices, moving = WvT
                bv_ap = bv_h[:]
                bv_bcast = bass.AP(
                    tensor=bv_ap.tensor,
                    offset=bv_ap.offset,
                    ap=[[0, 128], *bv_ap.ap],
                )
                nc.gpsimd.dma_start(out=bvb, in_=bv_bcast)
                wsb = []
                for kk in range(KD):
                    wt = wpool.tile([128, H], FP16, tag=f"w{kk}", name=f"w_v_{kk}")
                    weng = nc.gpsimd
                    wdma = weng.dma_start(out=wt, in_=wvT_h[kk * 128 : (kk + 1) * 128, :])
                    if anchor[0] is not None:
                        add_dep_helper(
                            wdma.ins, anchor[0], sync=False,
                            reason="wv prefetch behind k head chunks",
                        )
                    wsb.append(wt)
                for g in range(MS // 4):  # groups of 4 sk tiles = 512 cols of vT
                    vch = []
                    for kk in range(KD):
                        vt = astream.tile([128, 512], FP16, tag=f"a{kk}", name=f"vch_{g}_{kk}")
                        veng = nc.gpsimd
                        veng.dma_start(
                            out=vt,
                            in_=vT_h[
                                kk * 128 : (kk + 1) * 128, g * 512 : (g + 1) * 512
                            ],
                        )
                        vch.append(vt)
                    for mloc in range(4):
                        m = g * 4 + mloc
                        for n in range(NH):
                            ps = psum.tile([128, 512], FP32, tag="ps")
                            for kk in range(KD):
                                nc.tensor.matmul(
                                    ps,
                                    vch[kk][:, mloc * 128 : (mloc + 1) * 128],
                                    wsb[kk][:, n * 512 : (n + 1) * 512],
                                    start=(kk == 0),
                                    stop=(kk == KD - 1),
                                )
                            nc.vector.tensor_tensor(
                                V[m][:, n * 512 : (n + 1) * 512],
                                ps,
                                bvb[:, n * 512 : (n + 1) * 512],
                                mybir.AluOpType.add,
                            )

            # ---- phase 2+3: attention ----
            with (
                tc.tile_pool(name="wt", bufs=1) as wt_pool,
                tc.tile_pool(name="eblk", bufs=2) as epool,
                tc.tile_pool(name="sums", bufs=8) as spool,
                tc.tile_pool(name="ostage", bufs=4) as opool,
            ):
                WT = [wt_pool.tile([128, S], FP16, tag=f"wt{m}", name=f"WT{m}") for m in range(MS)]

                for m in range(MS):
                    eb = epool.tile([128, S], FP32, tag="eb")
                    for n in range(NS):
                        ps = psum.tile([128, 512], FP32, tag="ps")
                        for kk in range(MH):
                            nc.tensor.matmul(
                                ps,
                                KT[kk][:, m * 128 : (m + 1) * 128],
                                QT[kk][:, n * 512 : (n + 1) * 512],
                                start=(kk == 0),
                                stop=(kk == MH - 1),
                            )
                        nc.scalar.activation(
                            out=eb[:, n * 512 : (n + 1) * 512],
                            in_=ps,
                            func=mybir.ActivationFunctionType.Exp,
                            scale=SCALE,
                        )
                    ssum = spool.tile([128, 1], FP32, tag="s")
                    nc.vector.tensor_reduce(
                        out=ssum, in_=eb, axis=mybir.AxisListType.X,
                        op=mybir.AluOpType.add,
                    )
                    rcp = spool.tile([128, 1], FP32, tag="r")
                    nc.vector.reciprocal(rcp, ssum)
                    # bf16 normalized copy for the second bmm
                    nc.scalar.activation(
                        out=WT[m],
                        in_=eb,
                        func=mybir.ActivationFunctionType.Copy,
                        scale=rcp,
                    )
                    # f32 normalized row block -> DRAM (transposed attn_weights)
                    nc.vector.tensor_scalar_mul(eb, eb, rcp)
                    nc.gpsimd.dma_start(out=wT_h[m * 128 : (m + 1) * 128, :], in_=eb)

                # out tiles in groups of 8 PSUM banks with the sk-block loop (b)
                # hoisted to the middle: the WT[15]-dependent matmuls sit ~120
                # instructions deep, hiding the last softmax block's latency.
                otiles = [(mm, n) for mm in range(MS) for n in range(NH)]
                bounds = list(range(0, 28, 4)) + [28, 30]
                for gi, gstart in enumerate(bounds):
                    gend = bounds[gi + 1] if gi + 1 < len(bounds) else len(otiles)
                    group = otiles[gstart:gend]
                    g = gstart
                    pss = [
                        psum.tile([128, 512], FP32, tag="ps", name=f"ops_{g}_{j}")
                        for j in range(len(group))
                    ]
                    for b in range(MS):
                        for j, (mm, n) in enumerate(group):
                            nc.tensor.matmul(
                                pss[j],
                                WT[b][:, mm * 128 : (mm + 1) * 128],
                                V[b][:, n * 512 : (n + 1) * 512],
                                start=(b == 0),
                                stop=(b == MS - 1),
                            )
                    for j, (mm, n) in enumerate(group):
                        ost = opool.tile([128, 512], FP32, tag="o")
                        if j % 2 == 0:
                            nc.vector.tensor_copy(ost, pss[j])
                        else:
                            nc.scalar.activation(
                                out=ost, in_=pss[j],
                                func=mybir.ActivationFunctionType.Copy,
                            )
                        nc.scalar.dma_start(
                            out=out_h[mm * 128 : (mm + 1) * 128, n * 512 : (n + 1) * 512],
                            in_=ost,
                        )

    return nc


_GRAPH_CACHE = {}


def get_graph() -> bass.Bass:
    if "nc" not in _GRAPH_CACHE:
        _GRAPH_CACHE["nc"] = build_graph()
    return _GRAPH_CACHE["nc"]


def kernel(q, k, v, Wq, bq, Wk, bk, Wv, bv):
    from concourse.bass_utils import run_bass_kernel_spmd

    assert q.shape == (B, S, D)
    nc = get_graph()

    qT = np.ascontiguousarray(q.transpose(0, 2, 1), dtype=np.float32)
    kT = np.ascontiguousarray(k.transpose(0, 2, 1), dtype=np.float32)
    vT = np.ascontiguousarray(v.transpose(0, 2, 1), dtype=np.float32)
    wqT = np.ascontiguousarray(Wq.T, dtype=np.float32)
    wkT = np.ascontiguousarray(Wk.T, dtype=np.float32)
    wvT = np.ascontiguousarray(Wv.T, dtype=np.float32)
    bq = np.ascontiguousarray(bq, dtype=np.float32)
    bk = np.ascontiguousarray(bk, dtype=np.float32)
    bv = np.ascontiguousarray(bv, dtype=np.float32)

    in_maps = [
        dict(
            qT=qT[i], kT=kT[i], vT=vT[i],
            wqT=wqT, wkT=wkT, wvT=wvT,
            bq=bq, bk=bk, bv=bv,
        )
        for i in range(N_CORES)
    ]
    res = run_bass_kernel_spmd(nc, in_maps, core_ids=list(range(N_CORES)))
    attn_outputs = np.stack([res.results[i]["out"] for i in range(N_CORES)])
    wT = np.stack([res.results[i]["wT"] for i in range(N_CORES)])
    attn_weights = np.ascontiguousarray(wT.transpose(0, 2, 1))
    return attn_outputs, attn_weights


# revision 28
# speedup vs baseline: 1.0277x; 1.0012x over previous
"""Trainium2 Bass kernel for an attention block, data-parallel over batch on 8 NeuronCores.

Reference computation per batch b (softmax over the QUERY axis, axis=1):
    Q = q @ Wq.T + bq            [S, H]
    K = k @ Wk.T + bk            [S, H]
    V = v @ Wv.T + bv            [S, H]
    dot = Q @ K.T                [Sq, Sk]
    W = softmax(dot / sqrt(H), axis=0 over Sq)   (column softmax)
    out = W @ V                  [Sq, H]
    returns (out, W)

Sharding: B=8 batches -> 8 cores, one batch per core; weights replicated.
Host does layout only (transpose/slice/stack); all arithmetic on device.

Device dataflow per core (all contractions need the contracted dim on SBUF
partitions, so inputs are fed pre-transposed; all matmuls run in fp16 at the
full 1-column/cycle PE rate, f32 inputs converted by gpsimd cast-DMAs):
    QT[h, sq] = sum_d WqT[d, h].T-part @ qT[d, sq]
    KT[h, sk] likewise; V[sk, h] = vT tiles (stationary) x WvT (moving)
    dotT[sk, sq] = KT-tiles (stationary) x QT (moving)  -> PSUM
    eblk = exp(dotT / 32)  (ScalarE, from PSUM)  [sk_p, sq_f]
    rowsum over free axis (=sum over q) -> reciprocal -> normalize
    WT[sk, sq] written to DRAM f32 (host transposes back); fp16 copy kept for:
    out[sq, h] = sum_sk WT-tiles (stationary) x V (moving), b-hoisted in
    4-bank PSUM groups so the last softmax block's latency is hidden.
"""

import sys

if "/opt/trn_rl_repo" not in sys.path:
    sys.path.insert(0, "/opt/trn_rl_repo")

import numpy as np

import concourse.bass as bass
import concourse.mybir as mybir
import concourse.tile as tile
from concourse.tile_rust import add_dep_helper
from concourse.vector_clock import ScopedClock

B, S, D, H = 8, 2048, 1024, 1024
FP32 = mybir.dt.float32
FP32R = mybir.dt.float32r
BF16 = mybir.dt.bfloat16
FP16 = mybir.dt.float16
SCALE = 1.0 / 32.0  # 1/sqrt(H)

N_CORES = 8


class SplitDrainTileContext(tile.TileContext):
    """The walrus build in this container caps sync waits at 1 per
    instruction; Tile can assign several.  Split the extra waits onto
    preceding NoOp instructions on the same engine (program order =>
    identical semantics)."""

    MAX_WAITS = 1

    def _lower_ordered_insts(self, ordered):
        nsplit = 0
        for bb_name, insts in list(ordered.items()):
            new_insts = []
            for inst in insts:
                si = getattr(inst, "sync_info", None)
                if si is not None and si.on_wait and len(si.on_wait) > self.MAX_WAITS:
                    waits = list(si.on_wait)
                    for j, w in enumerate(waits[: -self.MAX_WAITS]):
                        nop = mybir.InstNoOp(name=f"{inst.name}-sw{j}")
                        nop.engine = inst.engine
                        nop.sync_info = mybir.SyncInfo(on_wait=[w], on_update=[])
                        self.nc.register_instruction(nop)
                        new_insts.append(nop)
                        nsplit += 1
                    si.on_wait = waits[-self.MAX_WAITS :]
                new_insts.append(inst)
            ordered[bb_name] = new_insts
        if nsplit:
            print(f"SplitDrainTileContext: split {nsplit} extra sync waits")
        return super()._lower_ordered_insts(ordered)

    def _drain_and_barrier(self, tick_clock, wait_clock):
        drain_inst = self.nc.sync.drain()
        wait_clock.add_sem_waits(
            drain_inst.ins, ScopedClock({None: tick_clock.global_clock})
        )
        si = drain_inst.ins.sync_info
        waits = list(si.on_wait) if si and si.on_wait else []
        if len(waits) > self.MAX_WAITS:
            si.on_wait = waits[: self.MAX_WAITS]
            rest = waits[self.MAX_WAITS :]
            for i in range(0, len(rest), self.MAX_WAITS):
                extra = self.nc.sync.drain()
                extra.ins.sync_info = mybir.SyncInfo(
                    on_wait=rest[i : i + self.MAX_WAITS], on_update=[]
                )
        self.nc.all_engine_barrier()
        assert self.sems is not None
        popped = self.nc._tile_sem_poison_stack.pop()
        assert popped is self._sem_poison
        self.nc.clear_and_free_semaphores(list(self.sems.allocated().values()))
        self.nc.all_engine_barrier()


def build_graph() -> bass.Bass:
    nc = bass.Bass("TRN2", target_bir_lowering=False, debug=False)

    qT_h = nc.declare_dram_parameter("qT", [D, S], FP32, isOutput=False)
    kT_h = nc.declare_dram_parameter("kT", [D, S], FP32, isOutput=False)
    vT_h = nc.declare_dram_parameter("vT", [D, S], FP32, isOutput=False)
    wqT_h = nc.declare_dram_parameter("wqT", [D, H], FP32, isOutput=False)
    wkT_h = nc.declare_dram_parameter("wkT", [D, H], FP32, isOutput=False)
    wvT_h = nc.declare_dram_parameter("wvT", [D, H], FP32, isOutput=False)
    bq_h = nc.declare_dram_parameter("bq", [H], FP32, isOutput=False)
    bk_h = nc.declare_dram_parameter("bk", [H], FP32, isOutput=False)
    bv_h = nc.declare_dram_parameter("bv", [H], FP32, isOutput=False)
    out_h = nc.declare_dram_parameter("out", [S, H], FP32, isOutput=True)
    wT_h = nc.declare_dram_parameter("wT", [S, S], FP32, isOutput=True)

    KD = D // 128  # 8 contraction tiles for d
    MH = H // 128  # 8 h tiles
    NS = S // 512  # 4 sq chunks of 512
    MS = S // 128  # 16 sk tiles
    NH = H // 512  # 2 h chunks of 512

    with SplitDrainTileContext(nc) as tc:
        with (
            tc.tile_pool(name="consts", bufs=1) as consts,
            tc.tile_pool(name="qt", bufs=1) as qt_pool,
            tc.tile_pool(name="kt", bufs=1) as kt_pool,
            tc.tile_pool(name="vv", bufs=1) as v_pool,
            tc.tile_pool(name="psum", bufs=8, space="PSUM") as psum,
        ):
            # ---- constants ----
            bq_sb = consts.tile([128, MH], FP32, tag="bq")
            nc.scalar.dma_start(out=bq_sb, in_=bq_h[:].rearrange("(m p) -> p m", p=128))
            bk_sb = consts.tile([128, MH], FP32, tag="bk")
            nc.scalar.dma_start(out=bk_sb, in_=bk_h[:].rearrange("(m p) -> p m", p=128))
            bvb = consts.tile([128, H], FP16, tag="bvb")

            # ---- persistent activation buffers ----
            QT = [qt_pool.tile([128, S], FP16, tag=f"qt{kk}", name=f"QT{kk}") for kk in range(MH)]
            KT = [kt_pool.tile([128, S], FP16, tag=f"kt{kk}", name=f"KT{kk}") for kk in range(MH)]
            V = [v_pool.tile([128, H], FP16, tag=f"v{m}", name=f"V{m}") for m in range(MS)]

            # ---- phase 1: projections ----
            with (
                tc.tile_pool(name="wpool", bufs=2) as wpool,
                tc.tile_pool(name="astream", bufs=3) as astream,
            ):
                anchor = [None]  # a mid-projection inst the NEXT projection's
                                 # weight prefetch must not overtake
                for proj, (aT_h, w_h, outT, b_sb) in enumerate(
                    (
                        (qT_h, wqT_h, QT, bq_sb),
                        (kT_h, wkT_h, KT, bk_sb),
                    )
                ):
                    wsb = []
                    if proj == 0:
                        # head: interleave chunk/weight DMA emission per kk so
                        # the first 2-kk burst's deps are the first ~1.5MB of
                        # transfers, not the last of 6MB.
                        ach0 = []
                        for kk in range(KD):
                            at = astream.tile(
                                [128, 512], FP16, tag=f"a{kk}", name=f"ach0_{kk}"
                            )
                            aeng = nc.gpsimd
                            aeng.dma_start(
                                out=at, in_=aT_h[kk * 128 : (kk + 1) * 128, 0:512]
                            )
                            ach0.append(at)
                            wt = wpool.tile(
                                [128, H], FP16, tag=f"w{kk}", name=f"w_{proj}_{kk}"
                            )
                            weng = nc.gpsimd
                            weng.dma_start(
                                out=wt, in_=w_h[kk * 128 : (kk + 1) * 128, :]
                            )
                            wsb.append(wt)
                    else:
                        for kk in range(KD):
                            wt = wpool.tile(
                                [128, H], FP16, tag=f"w{kk}", name=f"w_{proj}_{kk}"
                            )
                            weng = nc.gpsimd
                            wdma = weng.dma_start(
                                out=wt, in_=w_h[kk * 128 : (kk + 1) * 128, :]
                            )
                            if anchor[0] is not None:
                                add_dep_helper(
                                    wdma.ins, anchor[0], sync=False,
                                    reason="weight prefetch behind head chunks",
                                )
                            wsb.append(wt)
                    for n in range(NS):
                        if proj == 0 and n == 0:
                            ach = ach0
                        else:
                            ach = []
                            for kk in range(KD):
                                at = astream.tile([128, 512], FP16, tag=f"a{kk}")
                                aeng = nc.gpsimd
                                adma = aeng.dma_start(
                                    out=at,
                                    in_=aT_h[
                                        kk * 128 : (kk + 1) * 128, n * 512 : (n + 1) * 512
                                    ],
                                )
                                if n == 1 and kk == KD - 1:
                                    anchor[0] = adma.ins
                                ach.append(at)
                        if proj == 0 and n == 0:
                            # 2-kk bursts: first matmuls need only ach[0..1] +
                            # wq[0..1]; later kk pairs stream in behind.
                            pss = [
                                psum.tile([128, 512], FP32, tag="ps", name=f"hps{m}")
                                for m in range(MH)
                            ]
                            bursts = [(0,), (1,), (2, 3), (4, 5), (6, 7)]
                            for burst in bursts:
                                for m in range(MH):
                                    for kk in burst:
                                        nc.tensor.matmul(
                                            pss[m],
                                            wsb[kk][:, m * 128 : (m + 1) * 128],
                                            ach[kk],
                                            start=(kk == 0),
                                            stop=(kk == KD - 1),
                                        )
                            for m in range(MH):
                                nc.vector.tensor_scalar_add(
                                    outT[m][:, n * 512 : (n + 1) * 512],
                                    pss[m],
                                    b_sb[:, m : m + 1],
                                )
                            continue
                        for m in range(MH):
                            ps = psum.tile([128, 512], FP32, tag="ps")
                            for kk in range(KD):
                                nc.tensor.matmul(
                                    ps,
                                    wsb[kk][:, m * 128 : (m + 1) * 128],
                                    ach[kk],
                                    start=(kk == 0),
                                    stop=(kk == KD - 1),
                                )
                            nc.vector.tensor_scalar_add(
                                outT[m][:, n * 512 : (n + 1) * 512],
                                ps,
                                b_sb[:, m : m + 1],
                            )

                # V projection: stationary = vT chunk slices, moving = WvT
                bv_ap = bv_h[:]
                bv_bcast = bass.AP(
                    tensor=bv_ap.tensor,
                    offset=bv_ap.offset,
                    ap=[[0, 128], *bv_ap.ap],
                )
                nc.gpsimd.dma_start(out=bvb, in_=bv_bcast)
                wsb = []
                for kk in range(KD):
                    wt = wpool.tile([128, H], FP16, tag=f"w{kk}", name=f"w_v_{kk}")
                    weng = nc.gpsimd
                    wdma = weng.dma_start(out=wt, in_=wvT_h[kk * 128 : (kk + 1) * 128, :])
                    if anchor[0] is not None:
                        add_dep_helper(
                            wdma.ins, anchor[0], sync=False,
                            reason="wv prefetch behind k head chunks",
                        )
                    wsb.append(wt)
                for g in range(MS // 4):  # groups of 4 sk tiles = 512 cols of vT
                    vch = []
                    for kk in range(KD):
                        vt = astream.tile([128, 512], FP16, tag=f"a{kk}", name=f"vch_{g}_{kk}")
                        veng = nc.gpsimd
                        veng.dma_start(
                            out=vt,
                            in_=vT_h[
                                kk * 128 : (kk + 1) * 128, g * 512 : (g + 1) * 512
                            ],
                        )
                        vch.append(vt)
                    for mloc in range(4):
                        m = g * 4 + mloc
                        for n in range(NH):
                            ps = psum.tile([128, 512], FP32, tag="ps")
                            for kk in range(KD):
                                nc.tensor.matmul(
                                    ps,
                                    vch[kk][:, mloc * 128 : (mloc + 1) * 128],
                                    wsb[kk][:, n * 512 : (n + 1) * 512],
                                    start=(kk == 0),
                                    stop=(kk == KD - 1),
                                )
                            nc.vector.tensor_tensor(
                                V[m][:, n * 512 : (n + 1) * 512],
                                ps,
                                bvb[:, n * 512 : (n + 1) * 512],
                                mybir.AluOpType.add,
                            )

            # ---- phase 2+3: attention ----
            with (
                tc.tile_pool(name="wt", bufs=1) as wt_pool,
                tc.tile_pool(name="eblk", bufs=2) as epool,
                tc.tile_pool(name="sums", bufs=8) as spool,
                tc.tile_pool(name="ostage", bufs=4) as opool,
            ):
                WT = [wt_pool.tile([128, S], FP16, tag=f"wt{m}", name=f"WT{m}") for m in range(MS)]

                for m in range(MS):
                    eb = epool.tile([128, S], FP32, tag="eb")
                    for n in range(NS):
                        ps = psum.tile([128, 512], FP32, tag="ps")
                        for kk in range(MH):
                            nc.tensor.matmul(
                                ps,
                                KT[kk][:, m * 128 : (m + 1) * 128],
                                QT[kk][:, n * 512 : (n + 1) * 512],
                                start=(kk == 0),
                                stop=(kk == MH - 1),
                            )
                        nc.scalar.activation(
                            out=eb[:, n * 512 : (n + 1) * 512],
                            in_=ps,
                            func=mybir.ActivationFunctionType.Exp,
                            scale=SCALE,
                        )
                    ssum = spool.tile([128, 1], FP32, tag="s")
                    nc.vector.tensor_reduce(
                        out=ssum, in_=eb, axis=mybir.AxisListType.X,
                        op=mybir.AluOpType.add,
                    )
                    rcp = spool.tile([128, 1], FP32, tag="r")
                    nc.vector.reciprocal(rcp, ssum)
                    # bf16 normalized copy for the second bmm
                    nc.scalar.activation(
                        out=WT[m],
                        in_=eb,
                        func=mybir.ActivationFunctionType.Copy,
                        scale=rcp,
                    )
                    # f32 normalized row block -> DRAM (transposed attn_weights)
                    nc.vector.tensor_scalar_mul(eb, eb, rcp)
                    nc.gpsimd.dma_start(out=wT_h[m * 128 : (m + 1) * 128, :], in_=eb)

                # out tiles in groups of 8 PSUM banks with the sk-block loop (b)
                # hoisted to the middle: the WT[15]-dependent matmuls sit ~120
                # instructions deep, hiding the last softmax block's latency.
                otiles = [(mm, n) for mm in range(MS) for n in range(NH)]
                bounds = list(range(0, 24, 4)) + [24, 26, 28, 30]
                for gi, gstart in enumerate(bounds):
                    gend = bounds[gi + 1] if gi + 1 < len(bounds) else len(otiles)
                    group = otiles[gstart:gend]
                    g = gstart
                    pss = [
                        psum.tile([128, 512], FP32, tag="ps", name=f"ops_{g}_{j}")
                        for j in range(len(group))
                    ]
                    for b in range(MS):
                        for j, (mm, n) in enumerate(group):
                            nc.tensor.matmul(
                                pss[j],
                                WT[b][:, mm * 128 : (mm + 1) * 128],
                                V[b][:, n * 512 : (n + 1) * 512],
                                start=(b == 0),
                                stop=(b == MS - 1),
                            )
                    for j, (mm, n) in enumerate(group):
                        ost = opool.tile([128, 512], FP32, tag="o")
                        if j % 2 == 0:
                            nc.vector.tensor_copy(ost, pss[j])
                        else:
                            nc.scalar.activation(
                                out=ost, in_=pss[j],
                                func=mybir.ActivationFunctionType.Copy,
                            )
                        nc.scalar.dma_start(
                            out=out_h[mm * 128 : (mm + 1) * 128, n * 512 : (n + 1) * 512],
                            in_=ost,
                        )

    return nc


_GRAPH_CACHE = {}


def get_graph() -> bass.Bass:
    if "nc" not in _GRAPH_CACHE:
        _GRAPH_CACHE["nc"] = build_graph()
    return _GRAPH_CACHE["nc"]


def kernel(q, k, v, Wq, bq, Wk, bk, Wv, bv):
    from concourse.bass_utils import run_bass_kernel_spmd

    assert q.shape == (B, S, D)
    nc = get_graph()

    qT = np.ascontiguousarray(q.transpose(0, 2, 1), dtype=np.float32)
    kT = np.ascontiguousarray(k.transpose(0, 2, 1), dtype=np.float32)
    vT = np.ascontiguousarray(v.transpose(0, 2, 1), dtype=np.float32)
    wqT = np.ascontiguousarray(Wq.T, dtype=np.float32)
    wkT = np.ascontiguousarray(Wk.T, dtype=np.float32)
    wvT = np.ascontiguousarray(Wv.T, dtype=np.float32)
    bq = np.ascontiguousarray(bq, dtype=np.float32)
    bk = np.ascontiguousarray(bk, dtype=np.float32)
    bv = np.ascontiguousarray(bv, dtype=np.float32)

    in_maps = [
        dict(
            qT=qT[i], kT=kT[i], vT=vT[i],
            wqT=wqT, wkT=wkT, wvT=wvT,
            bq=bq, bk=bk, bv=bv,
        )
        for i in range(N_CORES)
    ]
    res = run_bass_kernel_spmd(nc, in_maps, core_ids=list(range(N_CORES)))
    attn_outputs = np.stack([res.results[i]["out"] for i in range(N_CORES)])
    wT = np.stack([res.results[i]["wT"] for i in range(N_CORES)])
    attn_weights = np.ascontiguousarray(wT.transpose(0, 2, 1))
    return attn_outputs, attn_weights
